# revision 18
# baseline (speedup 1.0000x reference)
"""Trainium2 Bass kernel for nn_BiLSTM_7928509628689.

Masked bidirectional LSTM over N=2048 ragged sequences (T=64, D=512, H=256),
returning concat of final fwd/bwd hidden states [N, 2H].

Strategy (8 NeuronCores, data-parallel over N, 256 seqs/core):
  * Sequences are globally sorted by length (desc) and dealt round-robin to
    cores, so all cores carry a near-identical length profile. All
    sequences are right-aligned in time (they END at the last step), so at
    step s only the V_s longest sequences are active. V_s is baked into
    the program: every matmul / ACT / DVE op at step s is trimmed to V_s
    columns. Mean length is ~T/2, so this halves the PE columns.
  * All state kept TRANSPOSED: hT/cT [H, Ns] folded into persistent
    [128, 2*Ns] tiles updated in place (never-yet-active columns stay 0).
  * Per step and direction, gates^T [4H, V_s] are built in one 4-bank PSUM
    tile (bank order g,i,f,o) by one accumulation group per bank:
       4 matmuls  W_ih^T chunks @ x_s chunks    (input projection)
       2 matmuls  W_hh^T chunks @ hT halves     (recurrence)
       1 matmul   [b; mask_coef] @ [ones; maskinv_s]  (bias + pad forcing)
    Operands bf16 (fp32 PSUM accumulation), K=128 for every matmul so
    LDWEIGHTS stays FWL-pipelined under the stream.
  * Wide steps (V_s >= DRV) run the recurrence matmuls in fp8e4m3
    DoubleRow mode: one K=256 matmul replaces the two K=128 bf16 ones,
    halving the streamed columns of the h-projection. The hidden state is
    then stored as fp8 (h8) written directly by the DVE h-update; cell
    state and activations stay bf16, so only the recurrent matmul operand
    is quantized (validated ~1.2e-2 rel err vs 2e-2 budget).
  * Pad forcing: columns included before their sequence's first step get
    -40 on the i/f/o pre-activations, so their state is forced to ~0 until
    the sequence starts; the final state at the last step is exactly the
    masked-LSTM output for both directions (bwd consumes the time-reversed
    sequence).
  * ACT: one tanh over the g bank, one sigmoid spanning the i,f,o banks,
    one tanh(c); DVE does the elementwise updates on exact active ranges.

kernel(**inputs) takes the FULL unsharded inputs and returns [2048, 512] f32.
"""
import numpy as np

import concourse.tile as tile
from concourse import bacc, mybir
from concourse.bass_utils import run_bass_kernel_spmd
import bass_rust

F32 = mybir.dt.float32
BF16 = mybir.dt.bfloat16
F8 = mybir.dt.float8e4
DRPM = mybir.MatmulPerfMode.DoubleRow
AF = mybir.ActivationFunctionType
OP = mybir.AluOpType

N, T, D, H = 2048, 64, 512, 256
NCORES = 8
NS = N // NCORES           # 256 sequences per core
FH = 4 * H                 # 1024 gate rows
KD = D // 128              # 4 x-projection K chunks
KH = H // 128              # 2 h-projection K chunks
FORCE = -40.0              # gate penalty for pad steps
MB = 8                     # mask rhs block (steps per mask DMA)
DIRS = ("f", "b")
C_BF16 = True              # cell state dtype (bf16 enables DVE 4x mode)
DRV = 160                  # min V_s for fp8 DoubleRow recurrence matmuls

# merged packed-band PSUM slot layouts: 16 ranges of R in a 16R slot,
# ordered [g_f,g_b | i_f,i_b | f_f,f_b | o_f,o_b]; entries are
# (dir, m-chunk, offset) with m-chunk pairs in h-unit order. R=128 gives
# two parity slots (v<=128); R=64 gives four (v<=64), which lets pass1
# run two steps ahead without colliding with in-flight ACT reads.
def _mk_slots(R, nbank):
    order = []
    for g in (4, 0, 2, 6):  # g, i, f, o
        for d in ("f", "b"):
            order += [(d, g), (d, g + 1)]
    per = len(order) // nbank
    return [
        [(d, m, (b * per + j) * R) for j, (d, m) in
         enumerate(order[b * per : (b + 1) * per])]
        for b in range(nbank)
    ]


SLOT128 = _mk_slots(128, 4)
SLOT64 = _mk_slots(64, 2)
DOFF = {"f": 0, "b": 512}  # dir offset into shared c2/h2 state tiles
# PSUM bank order within the [128, 4*512] gates tile; sigmoid spans i,f,o
BANK_MS = ((4, 5), (0, 1), (2, 3), (6, 7))   # g, i, f, o
BANK_OF = [b * 512 for b in range(4)]

_NC_CACHE = {}


def _inst(r):
    return getattr(r, "ins", r)


def _build(t_steps, V):
    import contextlib

    nc = bacc.Bacc("TRN2", target_bir_lowering=False, debug=False)

    # first step using fp8 DoubleRow recurrence (V monotone nondecreasing)
    s_dr = next((s for s in range(t_steps) if int(V[s]) >= DRV), t_steps)

    x_dram = {}
    wih_d, whh_d, whh8_d, bm_d, out_d = {}, {}, {}, {}, {}
    for d in DIRS:
        # x stored [t, 128, KD, NS]: (p, k) <-> input dim  dd = KD*p + k
        x_dram[d] = nc.dram_tensor(
            f"x{d}", [t_steps, 128, KD, NS], BF16, kind="ExternalInput"
        ).ap()
        wih_d[d] = nc.dram_tensor(
            f"wih{d}", [128, KD, FH], BF16, kind="ExternalInput"
        ).ap()
        whh_d[d] = nc.dram_tensor(
            f"whh{d}", [128, KH, FH], BF16, kind="ExternalInput"
        ).ap()
        whh8_d[d] = nc.dram_tensor(
            f"whh8{d}", [128, KH, FH], F8, kind="ExternalInput"
        ).ap()
        bm_d[d] = nc.dram_tensor(f"bm{d}", [128, FH], BF16, kind="ExternalInput").ap()
        out_d[d] = nc.dram_tensor(
            f"hT{d}", [128, KH * NS], BF16, kind="ExternalOutput"
        ).ap()
    mask_d = nc.dram_tensor(
        "maskrhs", [128, t_steps * NS], BF16, kind="ExternalInput"
    ).ap()

    with tile.TileContext(nc) as tc:
        with contextlib.ExitStack() as ctx:
            wpool = ctx.enter_context(tc.tile_pool(name="w", bufs=1))
            xpool = ctx.enter_context(tc.tile_pool(name="x", bufs=4))
            mpool = ctx.enter_context(tc.tile_pool(name="mask", bufs=2))
            spool = ctx.enter_context(tc.tile_pool(name="state", bufs=1))
            opool = ctx.enter_context(tc.tile_pool(name="outs", bufs=1))
            apool = ctx.enter_context(tc.tile_pool(name="acts", bufs=2))
            pspool = ctx.enter_context(tc.tile_pool(name="ps", bufs=1, space="PSUM"))

            # spread weight loads across engine DMA queues so they land in
            # parallel with the first x tiles (which use the sync queue);
            # wih is split by gate group so the first bank's weights (g)
            # arrive first and step 0 can start early
            wq = {"f": nc.scalar, "b": nc.gpsimd}
            wih_t, whh_t, whh8_t, bm_t = {}, {}, {}, {}
            for d in DIRS:
                wih_t[d] = wpool.tile([128, KD, FH], BF16, tag=f"wih_{d}", name=f"wih_{d}")
                for gm in (4, 0, 2, 6):  # bank order: g, i, f, o
                    gsl = slice(gm * 128, (gm + 2) * 128)
                    wq[d].dma_start(wih_t[d][:, :, gsl], wih_d[d][:, :, gsl])
                bm_t[d] = wpool.tile([128, FH], BF16, tag=f"bm_{d}", name=f"bm_{d}")
                wq[d].dma_start(bm_t[d][:], bm_d[d][:])
            for d in DIRS:
                whh_t[d] = wpool.tile([128, KH, FH], BF16, tag=f"whh_{d}", name=f"whh_{d}")
                wq[d].dma_start(whh_t[d][:], whh_d[d][:])
                whh8_t[d] = wpool.tile([128, KH, FH], F8, tag=f"whh8_{d}", name=f"whh8_{d}")
                wq[d].dma_start(whh8_t[d][:], whh8_d[d][:])

            # persistent state tiles, updated in place; inactive columns
            # stay zero from this init. Both dirs share one tile so the
            # packed band can run cross-dir elementwise ops on one view.
            CDT = BF16 if C_BF16 else F32
            h2 = spool.tile([128, 2 * KH * NS], BF16, tag="h2", name="h2")
            nc.vector.memset(h2[:], 0.0)
            c2 = spool.tile([128, 2 * KH * NS], CDT, tag="c2", name="c2")
            nc.vector.memset(c2[:], 0.0)
            # fp8 copy of the hidden state for DoubleRow recurrence steps
            h8 = spool.tile([128, 2 * KH * NS], F8, tag="h8", name="h8")
            nc.vector.memset(h8[:], 0.0)

            def h8v(d, v):
                return h8[:, DOFF[d] : DOFF[d] + 2 * NS].rearrange(
                    "p (two n) -> p two n", two=2
                )[:, :, :v]

            # single shared gates tile: unpacked steps use halves per dir,
            # packed steps use parity-alternating merged 2048 slots
            PS = pspool.tile([128, 2 * 4 * 512], F32, tag="ps", name="ps")
            ps_t = {"f": PS[:, 0:2048], "b": PS[:, 2048:4096]}

            # PE warm-up burst: dense dummy matmuls during the initial
            # weight/x DMA window so HAM reaches full clock before step 0.
            # memset on gpsimd: it issues its first instruction much earlier
            # than the vector engine, so the warmup burst starts sooner
            wrm = wpool.tile([128, 512], BF16, tag="warm", name="warm")
            nc.gpsimd.memset(wrm[:], 0.0)
            NWARM = 28
            for i in range(NWARM):
                nc.tensor.matmul(
                    PS[:, 0:512], wrm[:, 0:128], wrm[:],
                    start=(i == 0), stop=(i == NWARM - 1),
                )

            # --- step scheduling helpers -------------------------------
            xts_q, mt_q = {}, {}

            def fetch(s2):
                """Issue x/mask DMAs for step s2 (idempotent)."""
                if s2 >= t_steps or s2 in xts_q:
                    return
                blk = s2 // MB
                if blk not in mt_q:
                    mw = min(MB, t_steps - blk * MB) * NS
                    mt = mpool.tile([128, MB * NS], BF16, tag="m", name="mtile")
                    nc.scalar.dma_start(
                        mt[:, :mw], mask_d[:, blk * MB * NS : blk * MB * NS + mw]
                    )
                    mt_q[blk] = mt
                    for old in [b for b in mt_q if b < blk - 1]:
                        del mt_q[old]
                v = int(V[s2])
                xts = {}
                xq = {"f": nc.sync, "b": nc.gpsimd}
                for d in DIRS:
                    xts[d] = xpool.tile(
                        [128, KD, NS], BF16, tag=f"x_{d}", name=f"x_{d}"
                    )
                    xq[d].dma_start(xts[d][:, :, :v], x_dram[d][s2][:, :, :v])
                xts_q[s2] = xts

            def vvq(ap, q, v):
                return ap.rearrange("p (q n) -> p q n", q=q)[:, :, :v]

            def mrhs_of(s, v):
                mt = mt_q[s // MB]
                return mt[:, (s % MB) * NS : (s % MB) * NS + v]

            def playout(s):
                v = int(V[s])
                return v, 128, 2048 * (s % 2), SLOT128

            def p1_packed(s):
                """x-projection + bias into the parity slot of step s."""
                v, R, base, slots = playout(s)
                xts = xts_q[s]
                mrhs = mrhs_of(s, v)
                for bank in slots:
                    start_mm = None
                    for idx, (d, m, off) in enumerate(bank):
                        o_ap = PS[:, base + off : base + off + v]
                        msl = slice(m * 128, (m + 1) * 128)
                        r = nc.tensor.matmul(
                            o_ap, wih_t[d][:, 0, msl], xts[d][:, 0, :v],
                            start=(idx == 0), stop=False,
                        )
                        if idx == 0:
                            start_mm = _inst(r)
                        else:
                            bass_rust.add_dep_helper(
                                _inst(r), start_mm, sync=False,
                                reason="psum bank group order",
                            )
                        for k in range(1, KD):
                            nc.tensor.matmul(
                                o_ap, wih_t[d][:, k, msl], xts[d][:, k, :v],
                                start=False, stop=False,
                            )
                        nc.tensor.matmul(
                            o_ap, bm_t[d][:, msl], mrhs,
                            start=False, stop=False,
                        )

            def p2_act_packed(s):
                """recurrent projection + merged cross-dir ACT/DVE of step s."""
                v, R, base, slots = playout(s)

                def v2(ap, q):
                    return vvq(ap, q, v)

                for bank in slots:
                    for idx, (d, m, off) in enumerate(bank):
                        o_ap = PS[:, base + off : base + off + v]
                        msl = slice(m * 128, (m + 1) * 128)
                        for kk in range(KH):
                            nc.tensor.matmul(
                                o_ap,
                                whh_t[d][:, kk, msl],
                                h2[:, DOFF[d] + kk * NS : DOFF[d] + kk * NS + v],
                                start=False,
                                stop=(idx == len(bank) - 1 and kk == KH - 1),
                            )

                # cross-dir elementwise: ranges of R ordered
                # [x_f0, x_f1, x_b0, x_b1] per gate, h-unit aligned with
                # the 256-wide ranges of the shared c2/h2 state tiles.
                # The sigmoid is split if/o: sig_if only waits on the i/f
                # banks' recurrence matmuls (the o bank finishes ~200ns
                # later) and is 4 ranges shorter, so the c-update chain
                # starts earlier; sig_o runs in the ACT idle window while
                # the DVE does the cell update, well before hmul needs it.
                tg = apool.tile([128, 4 * R], BF16, tag=f"tg2_{R}", name="tg2")
                nc.scalar.activation(
                    v2(tg[:], 4), v2(PS[:, base : base + 4 * R], 4), AF.Tanh
                )
                si = apool.tile([128, 12 * R], BF16, tag=f"si2_{R}", name="si2")
                nc.scalar.activation(
                    v2(si[:, 0 : 8 * R], 8),
                    v2(PS[:, base + 4 * R : base + 12 * R], 8),
                    AF.Sigmoid,
                )
                nc.scalar.activation(
                    v2(si[:, 8 * R : 12 * R], 4),
                    v2(PS[:, base + 12 * R : base + 16 * R], 4),
                    AF.Sigmoid,
                )
                # cmul first: it only needs sig_if, while t1 also waits on
                # tanh_g
                nc.vector.tensor_tensor(
                    v2(c2[:], 4), v2(si[:, 4 * R : 8 * R], 4), v2(c2[:], 4), OP.mult
                )
                t1 = apool.tile([128, 4 * R], BF16, tag=f"t12_{R}", name="t12")
                nc.vector.tensor_tensor(
                    v2(t1[:], 4), v2(si[:, 0 : 4 * R], 4), v2(tg[:], 4), OP.mult
                )
                nc.vector.tensor_tensor(
                    v2(c2[:], 4), v2(c2[:], 4), v2(t1[:], 4), OP.add
                )
                tcn = apool.tile([128, 4 * R], BF16, tag=f"tc2_{R}", name="tc2")
                nc.scalar.activation(v2(tcn[:], 4), v2(c2[:], 4), AF.Tanh)
                nc.vector.tensor_tensor(
                    v2(h2[:], 4), v2(si[:, 8 * R : 12 * R], 4), v2(tcn[:], 4), OP.mult
                )
                if s + 1 >= s_dr:
                    # next step's recurrence runs in fp8 DoubleRow: refresh
                    # the fp8 state copy (cheap DVE copy via scalar engine
                    # stays off the critical path; use DVE mult-by-one? a
                    # plain tensor_tensor copy is simplest)
                    nc.vector.tensor_tensor(
                        v2(h8[:], 4), v2(h2[:], 4), v2(h2[:], 4), OP.bypass
                    )
                del xts_q[s]

            def band(s):
                return 2 if int(V[s]) <= 128 else 3

            # --- main loop: the packed prefix is software-pipelined.
            # pass1 runs ahead of the serial ACT/DVE chain: two steps in
            # the 4-parity v<=64 band, one step in the 2-parity band
            # (deeper lookahead there would WAR-collide with in-flight
            # ACT reads). The unpacked suffix overlaps via dir order. ----
            next_p1 = 0
            for s in range(t_steps):
                fetch(s)
                fetch(s + 1)
                fetch(s + 2)
                v = int(V[s])
                last = s == t_steps - 1
                use_dr = s >= s_dr and s > 0

                def v2(ap, q):
                    return vvq(ap, q, v)

                if band(s) != 3:
                    assert not last
                    while next_p1 <= s:
                        p1_packed(next_p1)
                        next_p1 += 1
                    p2_act_packed(s)
                    fetch(s + 3)
                    while (
                        next_p1 < t_steps
                        and band(next_p1) != 3
                        and next_p1 <= s + (2 if band(next_p1) == 1 else 1)
                    ):
                        p1_packed(next_p1)
                        next_p1 += 1
                    continue

                xts = xts_q[s]
                mrhs = mrhs_of(s, v)
                for d in DIRS:
                    xt = xts[d]
                    ps = ps_t[d]
                    cc = c2[:, DOFF[d] : DOFF[d] + 512]
                    hh = h2[:, DOFF[d] : DOFF[d] + 512]

                    banks = [
                        [(b * 512 + half * NS, BANK_MS[b][half]) for half in range(2)]
                        for b in range(4)
                    ]
                    tg_sl = ps[:, 0:512]
                    sifo_sl = ps[:, 512 : 4 * 512]

                    # pass 1: x-projection + bias/mask (independent of h),
                    # one accumulation group per physical bank
                    for regs in banks:
                        start_mm = None
                        for idx, (off, m) in enumerate(regs):
                            o_ap = ps[:, off : off + v]
                            msl = slice(m * 128, (m + 1) * 128)
                            r = nc.tensor.matmul(
                                o_ap, wih_t[d][:, 0, msl], xt[:, 0, :v],
                                start=(idx == 0), stop=False,
                            )
                            if idx == 0:
                                start_mm = _inst(r)
                            else:
                                # later regions rely on the bank-wide
                                # has_written clear done by the start matmul
                                bass_rust.add_dep_helper(
                                    _inst(r), start_mm, sync=False,
                                    reason="psum bank group order",
                                )
                            for k in range(1, KD):
                                nc.tensor.matmul(
                                    o_ap, wih_t[d][:, k, msl], xt[:, k, :v],
                                    start=False, stop=False,
                                )
                            nc.tensor.matmul(
                                o_ap, bm_t[d][:, msl], mrhs,
                                start=False, stop=False,
                            )

                    # pass 2: recurrent projection last, so the PE only
                    # stalls on h right before the gates complete
                    for regs in banks:
                        for idx, (off, m) in enumerate(regs):
                            o_ap = ps[:, off : off + v]
                            msl = slice(m * 128, (m + 1) * 128)
                            if use_dr:
                                nc.tensor.matmul(
                                    o_ap, whh8_t[d][:, :, msl], h8v(d, v),
                                    start=False, stop=(idx == len(regs) - 1),
                                    perf_mode=DRPM,
                                )
                            else:
                                for kk in range(KH):
                                    nc.tensor.matmul(
                                        o_ap,
                                        whh_t[d][:, kk, msl],
                                        h2[:, DOFF[d] + kk * NS : DOFF[d] + kk * NS + v],
                                        start=False,
                                        stop=(idx == len(regs) - 1 and kk == KH - 1),
                                    )

                    # ACT/DVE on strided multi-range views that skip the
                    # dead gaps between half-ranges
                    tg = apool.tile([128, 512], BF16, tag=f"tg_{d}", name=f"tg_{d}")
                    nc.scalar.activation(v2(tg[:], 2), v2(tg_sl, 2), AF.Tanh)
                    # one sigmoid spanning i, f, o; out layout is 6 ranges of
                    # 256 (i0,i1,f0,f1,o0,o1)
                    si = apool.tile([128, 3 * 512], BF16, tag=f"si_{d}", name=f"si_{d}")
                    nc.scalar.activation(v2(si[:], 6), v2(sifo_sl, 6), AF.Sigmoid)

                    t1 = apool.tile([128, 512], BF16, tag=f"t1_{d}", name=f"t1_{d}")
                    nc.vector.tensor_tensor(
                        v2(t1[:], 2), v2(si[:, 0:512], 2), v2(tg[:], 2), OP.mult
                    )
                    nc.vector.tensor_tensor(
                        v2(cc, 2), v2(si[:, 512:1024], 2), v2(cc, 2), OP.mult
                    )
                    nc.vector.tensor_tensor(
                        v2(cc, 2), v2(cc, 2), v2(t1[:], 2), OP.add
                    )
                    tcn = apool.tile([128, 512], BF16, tag=f"tc_{d}", name=f"tc_{d}")
                    nc.scalar.activation(v2(tcn[:], 2), v2(cc, 2), AF.Tanh)
                    if last:
                        hf = opool.tile([128, 512], BF16, tag=f"hout_{d}", name=f"hout_{d}")
                        nc.vector.tensor_tensor(hf[:], si[:, 1024:1536], tcn[:], OP.mult)
                        nc.sync.dma_start(out_d[d][:], hf[:])
                    elif s + 1 >= s_dr:
                        # next step reads the fp8 state: write h8 directly
                        nc.vector.tensor_tensor(
                            vvq(h8[:, DOFF[d] : DOFF[d] + 512], 2, v),
                            v2(si[:, 1024:1536], 2), v2(tcn[:], 2), OP.mult
                        )
                    else:
                        nc.vector.tensor_tensor(
                            v2(hh, 2), v2(si[:, 1024:1536], 2), v2(tcn[:], 2), OP.mult
                        )

    nc.compile()
    return nc


def _get_nc(t_steps, V):
    key = (t_steps, tuple(V))
    if key not in _NC_CACHE:
        _NC_CACHE[key] = _build(t_steps, V)
    return _NC_CACHE[key]


def _prep_weights(W_ih, W_hh, b):
    """lhsT layouts for one direction."""
    import ml_dtypes

    wdt = ml_dtypes.bfloat16
    wih = np.ascontiguousarray(
        W_ih.T.reshape(128, KD, FH).astype(wdt)
    )  # (p, k) <-> dd = KD*p + k
    whh = np.ascontiguousarray(
        W_hh.T.reshape(KH, 128, FH).transpose(1, 0, 2).astype(wdt)
    )  # (p, kk) <-> hrow = 128*kk + p
    whh8 = np.ascontiguousarray(
        whh.astype(np.float32).astype(ml_dtypes.float8_e4m3fn)
    )
    coef = np.zeros(FH, np.float32)
    coef[: 2 * H] = FORCE       # i, f gates
    coef[3 * H :] = FORCE       # o gate
    bm = np.zeros((128, FH), np.float32)
    bm[0] = b.astype(np.float32)
    bm[1] = coef
    bm = np.ascontiguousarray(bm.astype(wdt))
    return wih, whh, whh8, bm


def _prep_core(seqs_c, lens_c, t_steps):
    """Per-core device arrays. seqs_c [NS, T, D], lens_c [NS] (sorted desc)."""
    import ml_dtypes

    bf16 = ml_dtypes.bfloat16
    ns = seqs_c.shape[0]
    shift = t_steps - lens_c  # pad steps per sequence
    src_t = np.arange(t_steps)[None, :] - shift[:, None]      # [NS, t]
    valid = src_t >= 0
    gat = seqs_c[np.arange(ns)[:, None], np.clip(src_t, 0, T - 1)]
    xf = np.where(valid[..., None], gat, np.float32(0.0))     # right-aligned
    xb = seqs_c[:, t_steps - 1 :: -1, :]                      # time-reversed

    def to_dev(x_ntd):
        # [NS, t, D] -> [t, 128, KD, NS] with dd = KD*p + k
        xt = x_ntd.transpose(1, 2, 0).astype(bf16)            # [t, D, NS]
        return np.ascontiguousarray(xt.reshape(t_steps, 128, KD, ns))

    maskinv = (np.arange(t_steps)[:, None] < shift[None, :]).astype(np.float32)
    maskrhs = np.zeros((128, t_steps * ns), np.float32)
    maskrhs[0] = 1.0
    maskrhs[1] = maskinv.reshape(t_steps * ns)
    maskrhs = np.ascontiguousarray(maskrhs.astype(bf16))
    return {"xf": to_dev(xf), "xb": to_dev(xb), "maskrhs": maskrhs}


def _unfold(hT):
    """[128, KH*NS] device tile -> [NS, H] h matrix."""
    hT = np.asarray(hT, dtype=np.float32)
    h_rows = np.concatenate([hT[:, i * NS : (i + 1) * NS] for i in range(KH)], axis=0)
    return h_rows.T  # [NS, H]


def _run(inputs, trace=False, t_cap=None, **spmd_kwargs):
    import ml_dtypes

    all_embs = np.asarray(inputs["all_embs"], dtype=np.float32)
    lengths = np.asarray(inputs["lengths"]).astype(np.int64)
    starts = np.asarray(inputs["starts"]).astype(np.int64)

    if np.array_equal(starts, np.arange(N, dtype=np.int64) * T):
        seqs = all_embs.reshape(N, T, D)
    else:
        seqs = all_embs[starts[:, None] + np.arange(T)[None, :]]

    # global sort by length desc, deal round-robin to cores
    order = np.argsort(-lengths, kind="stable")
    t_steps = int(lengths.max())
    if t_cap is not None:
        t_steps = min(t_steps, t_cap)
    core_idx = [order[c::NCORES] for c in range(NCORES)]  # [NCORES][NS]

    # baked active widths: V_s = max over cores of #{len >= t_steps - s}
    Ls = np.stack([np.minimum(lengths[ci], t_steps) for ci in core_idx])  # [NC, NS]
    thr = t_steps - np.arange(t_steps)  # [t]
    V = (Ls[:, None, :] >= thr[None, :, None]).sum(-1).max(0)  # [t]
    V = np.maximum(V, 1)

    w = {}
    for d, (wi, wh, bb) in {
        "f": (inputs["W_ih_f"], inputs["W_hh_f"], inputs["b_f"]),
        "b": (inputs["W_ih_b"], inputs["W_hh_b"], inputs["b_b"]),
    }.items():
        w[d] = _prep_weights(
            np.asarray(wi, np.float32), np.asarray(wh, np.float32),
            np.asarray(bb, np.float32),
        )

    in_maps = []
    for ci in range(NCORES):
        idx = core_idx[ci]
        m = _prep_core(seqs[idx], np.minimum(lengths[idx], t_steps), t_steps)
        in_maps.append(
            {
                "xf": m["xf"], "xb": m["xb"], "maskrhs": m["maskrhs"],
                "wihf": w["f"][0], "whhf": w["f"][1], "whh8f": w["f"][2],
                "bmf": w["f"][3],
                "wihb": w["b"][0], "whhb": w["b"][1], "whh8b": w["b"][2],
                "bmb": w["b"][3],
            }
        )

    nc = _get_nc(t_steps, V)
    res = None
    for attempt in range(3):
        try:
            res = run_bass_kernel_spmd(
                nc, in_maps, core_ids=list(range(NCORES)), trace=trace,
                **spmd_kwargs
            )
            break
        except Exception:
            # rare transient NRT_EXEC_UNIT_UNRECOVERABLE right after a
            # fresh NEFF load; a plain re-execute has always recovered
            if attempt == 2:
                raise
            import time as _time

            _time.sleep(2.0)

    out = np.empty((N, 2 * H), np.float32)
    for ci in range(NCORES):
        out[core_idx[ci], :H] = _unfold(res.results[ci]["hTf"])
        out[core_idx[ci], H:] = _unfold(res.results[ci]["hTb"])
    return out, res


def kernel(**inputs) -> np.ndarray:
    out, _ = _run(inputs)
    return out


# revision 19
# speedup vs baseline: 1.0145x; 1.0145x over previous
"""Trainium2 Bass kernel for nn_BiLSTM_7928509628689.

Masked bidirectional LSTM over N=2048 ragged sequences (T=64, D=512, H=256),
returning concat of final fwd/bwd hidden states [N, 2H].

Strategy (8 NeuronCores, data-parallel over N, 256 seqs/core):
  * Sequences are globally sorted by length (desc) and dealt round-robin to
    cores, so all cores carry a near-identical length profile. All
    sequences are right-aligned in time (they END at the last step), so at
    step s only the V_s longest sequences are active. V_s is baked into
    the program: every matmul / ACT / DVE op at step s is trimmed to V_s
    columns. Mean length is ~T/2, so this halves the PE columns.
  * All state kept TRANSPOSED: hT/cT [H, Ns] folded into persistent
    [128, 2*Ns] tiles updated in place (never-yet-active columns stay 0).
  * Per step and direction, gates^T [4H, V_s] are built in one 4-bank PSUM
    tile (bank order g,i,f,o) by one accumulation group per bank:
       4 matmuls  W_ih^T chunks @ x_s chunks    (input projection)
       2 matmuls  W_hh^T chunks @ hT halves     (recurrence)
       1 matmul   [b; mask_coef] @ [ones; maskinv_s]  (bias + pad forcing)
    Operands bf16 (fp32 PSUM accumulation), K=128 for every matmul so
    LDWEIGHTS stays FWL-pipelined under the stream.
  * Wide steps (V_s >= DRV) run the recurrence matmuls in fp8e4m3
    DoubleRow mode: one K=256 matmul replaces the two K=128 bf16 ones,
    halving the streamed columns of the h-projection. The hidden state is
    then stored as fp8 (h8) written directly by the DVE h-update; cell
    state and activations stay bf16, so only the recurrent matmul operand
    is quantized (validated ~1.2e-2 rel err vs 2e-2 budget).
  * Pad forcing: columns included before their sequence's first step get
    -40 on the i/f/o pre-activations, so their state is forced to ~0 until
    the sequence starts; the final state at the last step is exactly the
    masked-LSTM output for both directions (bwd consumes the time-reversed
    sequence).
  * ACT: one tanh over the g bank, one sigmoid spanning the i,f,o banks,
    one tanh(c); DVE does the elementwise updates on exact active ranges.

kernel(**inputs) takes the FULL unsharded inputs and returns [2048, 512] f32.
"""
import numpy as np

import concourse.tile as tile
from concourse import bacc, mybir
from concourse.bass_utils import run_bass_kernel_spmd
import bass_rust

F32 = mybir.dt.float32
BF16 = mybir.dt.bfloat16
F8 = mybir.dt.float8e4
DRPM = mybir.MatmulPerfMode.DoubleRow
AF = mybir.ActivationFunctionType
OP = mybir.AluOpType

N, T, D, H = 2048, 64, 512, 256
NCORES = 8
NS = N // NCORES           # 256 sequences per core
FH = 4 * H                 # 1024 gate rows
KD = D // 128              # 4 x-projection K chunks
KH = H // 128              # 2 h-projection K chunks
FORCE = -40.0              # gate penalty for pad steps
MB = 8                     # mask rhs block (steps per mask DMA)
DIRS = ("f", "b")
C_BF16 = True              # cell state dtype (bf16 enables DVE 4x mode)
DRV = 160                  # min V_s for fp8 DoubleRow recurrence matmuls

# merged packed-band PSUM slot layouts: 16 ranges of R in a 16R slot,
# ordered [g_f,g_b | i_f,i_b | f_f,f_b | o_f,o_b]; entries are
# (dir, m-chunk, offset) with m-chunk pairs in h-unit order. R=128 gives
# two parity slots (v<=128); R=64 gives four (v<=64), which lets pass1
# run two steps ahead without colliding with in-flight ACT reads.
def _mk_slots(R, nbank):
    order = []
    for g in (4, 0, 2, 6):  # g, i, f, o
        for d in ("f", "b"):
            order += [(d, g), (d, g + 1)]
    per = len(order) // nbank
    return [
        [(d, m, (b * per + j) * R) for j, (d, m) in
         enumerate(order[b * per : (b + 1) * per])]
        for b in range(nbank)
    ]


SLOT128 = _mk_slots(128, 4)
SLOT64 = _mk_slots(64, 2)
DOFF = {"f": 0, "b": 512}  # dir offset into shared c2/h2 state tiles
# PSUM bank order within the [128, 4*512] gates tile; sigmoid spans i,f,o
BANK_MS = ((4, 5), (0, 1), (2, 3), (6, 7))   # g, i, f, o
BANK_OF = [b * 512 for b in range(4)]

_NC_CACHE = {}


def _inst(r):
    return getattr(r, "ins", r)


def _build(t_steps, V):
    import contextlib

    nc = bacc.Bacc("TRN2", target_bir_lowering=False, debug=False)

    # first step using fp8 DoubleRow recurrence (V monotone nondecreasing)
    s_dr = next((s for s in range(t_steps) if int(V[s]) >= DRV), t_steps)

    x_dram = {}
    wih_d, whh_d, whh8_d, bm_d, out_d = {}, {}, {}, {}, {}
    for d in DIRS:
        # x stored [t, 128, KD, NS]: (p, k) <-> input dim  dd = KD*p + k
        x_dram[d] = nc.dram_tensor(
            f"x{d}", [t_steps, 128, KD, NS], BF16, kind="ExternalInput"
        ).ap()
        wih_d[d] = nc.dram_tensor(
            f"wih{d}", [128, KD, FH], BF16, kind="ExternalInput"
        ).ap()
        whh_d[d] = nc.dram_tensor(
            f"whh{d}", [128, KH, FH], BF16, kind="ExternalInput"
        ).ap()
        whh8_d[d] = nc.dram_tensor(
            f"whh8{d}", [128, KH, FH], F8, kind="ExternalInput"
        ).ap()
        bm_d[d] = nc.dram_tensor(f"bm{d}", [128, FH], BF16, kind="ExternalInput").ap()
        out_d[d] = nc.dram_tensor(
            f"hT{d}", [128, KH * NS], BF16, kind="ExternalOutput"
        ).ap()
    mask_d = nc.dram_tensor(
        "maskrhs", [128, t_steps * NS], BF16, kind="ExternalInput"
    ).ap()

    with tile.TileContext(nc) as tc:
        with contextlib.ExitStack() as ctx:
            wpool = ctx.enter_context(tc.tile_pool(name="w", bufs=1))
            xpool = ctx.enter_context(tc.tile_pool(name="x", bufs=4))
            mpool = ctx.enter_context(tc.tile_pool(name="mask", bufs=2))
            spool = ctx.enter_context(tc.tile_pool(name="state", bufs=1))
            opool = ctx.enter_context(tc.tile_pool(name="outs", bufs=1))
            apool = ctx.enter_context(tc.tile_pool(name="acts", bufs=2))
            pspool = ctx.enter_context(tc.tile_pool(name="ps", bufs=1, space="PSUM"))

            # spread weight loads across engine DMA queues so they land in
            # parallel with the first x tiles (which use the sync queue);
            # wih is split by gate group so the first bank's weights (g)
            # arrive first and step 0 can start early
            wq = {"f": nc.scalar, "b": nc.gpsimd}
            wih_t, whh_t, whh8_t, bm_t = {}, {}, {}, {}
            for d in DIRS:
                wih_t[d] = wpool.tile([128, KD, FH], BF16, tag=f"wih_{d}", name=f"wih_{d}")
                for gm in (4, 0, 2, 6):  # bank order: g, i, f, o
                    gsl = slice(gm * 128, (gm + 2) * 128)
                    wq[d].dma_start(wih_t[d][:, :, gsl], wih_d[d][:, :, gsl])
                bm_t[d] = wpool.tile([128, FH], BF16, tag=f"bm_{d}", name=f"bm_{d}")
                wq[d].dma_start(bm_t[d][:], bm_d[d][:])
            for d in DIRS:
                whh_t[d] = wpool.tile([128, KH, FH], BF16, tag=f"whh_{d}", name=f"whh_{d}")
                wq[d].dma_start(whh_t[d][:], whh_d[d][:])
                whh8_t[d] = wpool.tile([128, KH, FH], F8, tag=f"whh8_{d}", name=f"whh8_{d}")
                wq[d].dma_start(whh8_t[d][:], whh8_d[d][:])

            # persistent state tiles, updated in place; inactive columns
            # stay zero from this init. Both dirs share one tile so the
            # packed band can run cross-dir elementwise ops on one view.
            CDT = BF16 if C_BF16 else F32
            h2 = spool.tile([128, 2 * KH * NS], BF16, tag="h2", name="h2")
            nc.vector.memset(h2[:], 0.0)
            c2 = spool.tile([128, 2 * KH * NS], CDT, tag="c2", name="c2")
            nc.vector.memset(c2[:], 0.0)
            # fp8 copy of the hidden state for DoubleRow recurrence steps
            h8 = spool.tile([128, 2 * KH * NS], F8, tag="h8", name="h8")
            nc.vector.memset(h8[:], 0.0)

            def h8v(d, v):
                return h8[:, DOFF[d] : DOFF[d] + 2 * NS].rearrange(
                    "p (two n) -> p two n", two=2
                )[:, :, :v]

            # single shared gates tile: unpacked steps use halves per dir,
            # packed steps use parity-alternating merged 2048 slots
            PS = pspool.tile([128, 2 * 4 * 512], F32, tag="ps", name="ps")
            ps_t = {"f": PS[:, 0:2048], "b": PS[:, 2048:4096]}

            # PE warm-up burst: dense dummy matmuls during the initial
            # weight/x DMA window so HAM reaches full clock before step 0.
            wrm = wpool.tile([128, 512], BF16, tag="warm", name="warm")
            nc.vector.memset(wrm[:], 0.0)
            NWARM = 28
            for i in range(NWARM):
                nc.tensor.matmul(
                    PS[:, 0:512], wrm[:, 0:128], wrm[:],
                    start=(i == 0), stop=(i == NWARM - 1),
                )

            # --- step scheduling helpers -------------------------------
            xts_q, mt_q = {}, {}

            def fetch(s2):
                """Issue x/mask DMAs for step s2 (idempotent)."""
                if s2 >= t_steps or s2 in xts_q:
                    return
                blk = s2 // MB
                if blk not in mt_q:
                    mw = min(MB, t_steps - blk * MB) * NS
                    mt = mpool.tile([128, MB * NS], BF16, tag="m", name="mtile")
                    nc.sync.dma_start(
                        mt[:, :mw], mask_d[:, blk * MB * NS : blk * MB * NS + mw]
                    )
                    mt_q[blk] = mt
                    for old in [b for b in mt_q if b < blk - 1]:
                        del mt_q[old]
                v = int(V[s2])
                xts = {}
                for d in DIRS:
                    xts[d] = xpool.tile(
                        [128, KD, NS], BF16, tag=f"x_{d}", name=f"x_{d}"
                    )
                    nc.sync.dma_start(xts[d][:, :, :v], x_dram[d][s2][:, :, :v])
                xts_q[s2] = xts

            def vvq(ap, q, v):
                return ap.rearrange("p (q n) -> p q n", q=q)[:, :, :v]

            def mrhs_of(s, v):
                mt = mt_q[s // MB]
                return mt[:, (s % MB) * NS : (s % MB) * NS + v]

            def playout(s):
                v = int(V[s])
                return v, 128, 2048 * (s % 2), SLOT128

            def p1_packed(s):
                """x-projection + bias into the parity slot of step s."""
                v, R, base, slots = playout(s)
                xts = xts_q[s]
                mrhs = mrhs_of(s, v)
                for bank in slots:
                    start_mm = None
                    for idx, (d, m, off) in enumerate(bank):
                        o_ap = PS[:, base + off : base + off + v]
                        msl = slice(m * 128, (m + 1) * 128)
                        r = nc.tensor.matmul(
                            o_ap, wih_t[d][:, 0, msl], xts[d][:, 0, :v],
                            start=(idx == 0), stop=False,
                        )
                        if idx == 0:
                            start_mm = _inst(r)
                        else:
                            bass_rust.add_dep_helper(
                                _inst(r), start_mm, sync=False,
                                reason="psum bank group order",
                            )
                        for k in range(1, KD):
                            nc.tensor.matmul(
                                o_ap, wih_t[d][:, k, msl], xts[d][:, k, :v],
                                start=False, stop=False,
                            )
                        nc.tensor.matmul(
                            o_ap, bm_t[d][:, msl], mrhs,
                            start=False, stop=False,
                        )

            def p2_act_packed(s):
                """recurrent projection + merged cross-dir ACT/DVE of step s."""
                v, R, base, slots = playout(s)

                def v2(ap, q):
                    return vvq(ap, q, v)

                for bank in slots:
                    for idx, (d, m, off) in enumerate(bank):
                        o_ap = PS[:, base + off : base + off + v]
                        msl = slice(m * 128, (m + 1) * 128)
                        for kk in range(KH):
                            nc.tensor.matmul(
                                o_ap,
                                whh_t[d][:, kk, msl],
                                h2[:, DOFF[d] + kk * NS : DOFF[d] + kk * NS + v],
                                start=False,
                                stop=(idx == len(bank) - 1 and kk == KH - 1),
                            )

                # cross-dir elementwise: ranges of R ordered
                # [x_f0, x_f1, x_b0, x_b1] per gate, h-unit aligned with
                # the 256-wide ranges of the shared c2/h2 state tiles.
                # The sigmoid is split if/o: sig_if only waits on the i/f
                # banks' recurrence matmuls (the o bank finishes ~200ns
                # later) and is 4 ranges shorter, so the c-update chain
                # starts earlier; sig_o runs in the ACT idle window while
                # the DVE does the cell update, well before hmul needs it.
                tg = apool.tile([128, 4 * R], BF16, tag=f"tg2_{R}", name="tg2")
                nc.scalar.activation(
                    v2(tg[:], 4), v2(PS[:, base : base + 4 * R], 4), AF.Tanh
                )
                si = apool.tile([128, 12 * R], BF16, tag=f"si2_{R}", name="si2")
                nc.scalar.activation(
                    v2(si[:, 0 : 8 * R], 8),
                    v2(PS[:, base + 4 * R : base + 12 * R], 8),
                    AF.Sigmoid,
                )
                nc.scalar.activation(
                    v2(si[:, 8 * R : 12 * R], 4),
                    v2(PS[:, base + 12 * R : base + 16 * R], 4),
                    AF.Sigmoid,
                )
                # cmul first: it only needs sig_if, while t1 also waits on
                # tanh_g
                nc.vector.tensor_tensor(
                    v2(c2[:], 4), v2(si[:, 4 * R : 8 * R], 4), v2(c2[:], 4), OP.mult
                )
                t1 = apool.tile([128, 4 * R], BF16, tag=f"t12_{R}", name="t12")
                nc.vector.tensor_tensor(
                    v2(t1[:], 4), v2(si[:, 0 : 4 * R], 4), v2(tg[:], 4), OP.mult
                )
                nc.vector.tensor_tensor(
                    v2(c2[:], 4), v2(c2[:], 4), v2(t1[:], 4), OP.add
                )
                tcn = apool.tile([128, 4 * R], BF16, tag=f"tc2_{R}", name="tc2")
                nc.scalar.activation(v2(tcn[:], 4), v2(c2[:], 4), AF.Tanh)
                nc.vector.tensor_tensor(
                    v2(h2[:], 4), v2(si[:, 8 * R : 12 * R], 4), v2(tcn[:], 4), OP.mult
                )
                if s + 1 >= s_dr:
                    # next step's recurrence runs in fp8 DoubleRow: refresh
                    # the fp8 state copy (cheap DVE copy via scalar engine
                    # stays off the critical path; use DVE mult-by-one? a
                    # plain tensor_tensor copy is simplest)
                    nc.vector.tensor_tensor(
                        v2(h8[:], 4), v2(h2[:], 4), v2(h2[:], 4), OP.bypass
                    )
                del xts_q[s]

            def band(s):
                return 2 if int(V[s]) <= 128 else 3

            # --- main loop: the packed prefix is software-pipelined.
            # pass1 runs ahead of the serial ACT/DVE chain: two steps in
            # the 4-parity v<=64 band, one step in the 2-parity band
            # (deeper lookahead there would WAR-collide with in-flight
            # ACT reads). The unpacked suffix overlaps via dir order. ----
            next_p1 = 0
            for s in range(t_steps):
                fetch(s)
                fetch(s + 1)
                fetch(s + 2)
                v = int(V[s])
                last = s == t_steps - 1
                use_dr = s >= s_dr and s > 0

                def v2(ap, q):
                    return vvq(ap, q, v)

                if band(s) != 3:
                    assert not last
                    while next_p1 <= s:
                        p1_packed(next_p1)
                        next_p1 += 1
                    p2_act_packed(s)
                    fetch(s + 3)
                    while (
                        next_p1 < t_steps
                        and band(next_p1) != 3
                        and next_p1 <= s + (2 if band(next_p1) == 1 else 1)
                    ):
                        p1_packed(next_p1)
                        next_p1 += 1
                    continue

                xts = xts_q[s]
                mrhs = mrhs_of(s, v)
                for d in DIRS:
                    xt = xts[d]
                    ps = ps_t[d]
                    cc = c2[:, DOFF[d] : DOFF[d] + 512]
                    hh = h2[:, DOFF[d] : DOFF[d] + 512]

                    banks = [
                        [(b * 512 + half * NS, BANK_MS[b][half]) for half in range(2)]
                        for b in range(4)
                    ]
                    tg_sl = ps[:, 0:512]
                    sifo_sl = ps[:, 512 : 4 * 512]

                    # pass 1: x-projection + bias/mask (independent of h),
                    # one accumulation group per physical bank
                    for regs in banks:
                        start_mm = None
                        for idx, (off, m) in enumerate(regs):
                            o_ap = ps[:, off : off + v]
                            msl = slice(m * 128, (m + 1) * 128)
                            r = nc.tensor.matmul(
                                o_ap, wih_t[d][:, 0, msl], xt[:, 0, :v],
                                start=(idx == 0), stop=False,
                            )
                            if idx == 0:
                                start_mm = _inst(r)
                            else:
                                # later regions rely on the bank-wide
                                # has_written clear done by the start matmul
                                bass_rust.add_dep_helper(
                                    _inst(r), start_mm, sync=False,
                                    reason="psum bank group order",
                                )
                            for k in range(1, KD):
                                nc.tensor.matmul(
                                    o_ap, wih_t[d][:, k, msl], xt[:, k, :v],
                                    start=False, stop=False,
                                )
                            nc.tensor.matmul(
                                o_ap, bm_t[d][:, msl], mrhs,
                                start=False, stop=False,
                            )

                    # pass 2: recurrent projection last, so the PE only
                    # stalls on h right before the gates complete
                    for regs in banks:
                        for idx, (off, m) in enumerate(regs):
                            o_ap = ps[:, off : off + v]
                            msl = slice(m * 128, (m + 1) * 128)
                            if use_dr:
                                nc.tensor.matmul(
                                    o_ap, whh8_t[d][:, :, msl], h8v(d, v),
                                    start=False, stop=(idx == len(regs) - 1),
                                    perf_mode=DRPM,
                                )
                            else:
                                for kk in range(KH):
                                    nc.tensor.matmul(
                                        o_ap,
                                        whh_t[d][:, kk, msl],
                                        h2[:, DOFF[d] + kk * NS : DOFF[d] + kk * NS + v],
                                        start=False,
                                        stop=(idx == len(regs) - 1 and kk == KH - 1),
                                    )

                    # ACT/DVE on strided multi-range views that skip the
                    # dead gaps between half-ranges
                    tg = apool.tile([128, 512], BF16, tag=f"tg_{d}", name=f"tg_{d}")
                    nc.scalar.activation(v2(tg[:], 2), v2(tg_sl, 2), AF.Tanh)
                    # one sigmoid spanning i, f, o; out layout is 6 ranges of
                    # 256 (i0,i1,f0,f1,o0,o1)
                    si = apool.tile([128, 3 * 512], BF16, tag=f"si_{d}", name=f"si_{d}")
                    nc.scalar.activation(v2(si[:], 6), v2(sifo_sl, 6), AF.Sigmoid)

                    t1 = apool.tile([128, 512], BF16, tag=f"t1_{d}", name=f"t1_{d}")
                    nc.vector.tensor_tensor(
                        v2(t1[:], 2), v2(si[:, 0:512], 2), v2(tg[:], 2), OP.mult
                    )
                    nc.vector.tensor_tensor(
                        v2(cc, 2), v2(si[:, 512:1024], 2), v2(cc, 2), OP.mult
                    )
                    nc.vector.tensor_tensor(
                        v2(cc, 2), v2(cc, 2), v2(t1[:], 2), OP.add
                    )
                    tcn = apool.tile([128, 512], BF16, tag=f"tc_{d}", name=f"tc_{d}")
                    nc.scalar.activation(v2(tcn[:], 2), v2(cc, 2), AF.Tanh)
                    if last:
                        hf = opool.tile([128, 512], BF16, tag=f"hout_{d}", name=f"hout_{d}")
                        nc.vector.tensor_tensor(hf[:], si[:, 1024:1536], tcn[:], OP.mult)
                        nc.sync.dma_start(out_d[d][:], hf[:])
                    elif s + 1 >= s_dr:
                        # next step reads the fp8 state: write h8 directly
                        nc.vector.tensor_tensor(
                            vvq(h8[:, DOFF[d] : DOFF[d] + 512], 2, v),
                            v2(si[:, 1024:1536], 2), v2(tcn[:], 2), OP.mult
                        )
                    else:
                        nc.vector.tensor_tensor(
                            v2(hh, 2), v2(si[:, 1024:1536], 2), v2(tcn[:], 2), OP.mult
                        )

    nc.compile()
    return nc


def _get_nc(t_steps, V):
    key = (t_steps, tuple(V))
    if key not in _NC_CACHE:
        _NC_CACHE[key] = _build(t_steps, V)
    return _NC_CACHE[key]


def _prep_weights(W_ih, W_hh, b):
    """lhsT layouts for one direction."""
    import ml_dtypes

    wdt = ml_dtypes.bfloat16
    wih = np.ascontiguousarray(
        W_ih.T.reshape(128, KD, FH).astype(wdt)
    )  # (p, k) <-> dd = KD*p + k
    whh = np.ascontiguousarray(
        W_hh.T.reshape(KH, 128, FH).transpose(1, 0, 2).astype(wdt)
    )  # (p, kk) <-> hrow = 128*kk + p
    whh8 = np.ascontiguousarray(
        whh.astype(np.float32).astype(ml_dtypes.float8_e4m3fn)
    )
    coef = np.zeros(FH, np.float32)
    coef[: 2 * H] = FORCE       # i, f gates
    coef[3 * H :] = FORCE       # o gate
    bm = np.zeros((128, FH), np.float32)
    bm[0] = b.astype(np.float32)
    bm[1] = coef
    bm = np.ascontiguousarray(bm.astype(wdt))
    return wih, whh, whh8, bm


def _prep_core(seqs_c, lens_c, t_steps):
    """Per-core device arrays. seqs_c [NS, T, D], lens_c [NS] (sorted desc)."""
    import ml_dtypes

    bf16 = ml_dtypes.bfloat16
    ns = seqs_c.shape[0]
    shift = t_steps - lens_c  # pad steps per sequence
    src_t = np.arange(t_steps)[None, :] - shift[:, None]      # [NS, t]
    valid = src_t >= 0
    gat = seqs_c[np.arange(ns)[:, None], np.clip(src_t, 0, T - 1)]
    xf = np.where(valid[..., None], gat, np.float32(0.0))     # right-aligned
    xb = seqs_c[:, t_steps - 1 :: -1, :]                      # time-reversed

    def to_dev(x_ntd):
        # [NS, t, D] -> [t, 128, KD, NS] with dd = KD*p + k
        xt = x_ntd.transpose(1, 2, 0).astype(bf16)            # [t, D, NS]
        return np.ascontiguousarray(xt.reshape(t_steps, 128, KD, ns))

    maskinv = (np.arange(t_steps)[:, None] < shift[None, :]).astype(np.float32)
    maskrhs = np.zeros((128, t_steps * ns), np.float32)
    maskrhs[0] = 1.0
    maskrhs[1] = maskinv.reshape(t_steps * ns)
    maskrhs = np.ascontiguousarray(maskrhs.astype(bf16))
    return {"xf": to_dev(xf), "xb": to_dev(xb), "maskrhs": maskrhs}


def _unfold(hT):
    """[128, KH*NS] device tile -> [NS, H] h matrix."""
    hT = np.asarray(hT, dtype=np.float32)
    h_rows = np.concatenate([hT[:, i * NS : (i + 1) * NS] for i in range(KH)], axis=0)
    return h_rows.T  # [NS, H]


def _run(inputs, trace=False, t_cap=None, **spmd_kwargs):
    import ml_dtypes

    all_embs = np.asarray(inputs["all_embs"], dtype=np.float32)
    lengths = np.asarray(inputs["lengths"]).astype(np.int64)
    starts = np.asarray(inputs["starts"]).astype(np.int64)

    if np.array_equal(starts, np.arange(N, dtype=np.int64) * T):
        seqs = all_embs.reshape(N, T, D)
    else:
        seqs = all_embs[starts[:, None] + np.arange(T)[None, :]]

    # global sort by length desc, deal round-robin to cores
    order = np.argsort(-lengths, kind="stable")
    t_steps = int(lengths.max())
    if t_cap is not None:
        t_steps = min(t_steps, t_cap)
    core_idx = [order[c::NCORES] for c in range(NCORES)]  # [NCORES][NS]

    # baked active widths: V_s = max over cores of #{len >= t_steps - s}
    Ls = np.stack([np.minimum(lengths[ci], t_steps) for ci in core_idx])  # [NC, NS]
    thr = t_steps - np.arange(t_steps)  # [t]
    V = (Ls[:, None, :] >= thr[None, :, None]).sum(-1).max(0)  # [t]
    V = np.maximum(V, 1)

    w = {}
    for d, (wi, wh, bb) in {
        "f": (inputs["W_ih_f"], inputs["W_hh_f"], inputs["b_f"]),
        "b": (inputs["W_ih_b"], inputs["W_hh_b"], inputs["b_b"]),
    }.items():
        w[d] = _prep_weights(
            np.asarray(wi, np.float32), np.asarray(wh, np.float32),
            np.asarray(bb, np.float32),
        )

    in_maps = []
    for ci in range(NCORES):
        idx = core_idx[ci]
        m = _prep_core(seqs[idx], np.minimum(lengths[idx], t_steps), t_steps)
        in_maps.append(
            {
                "xf": m["xf"], "xb": m["xb"], "maskrhs": m["maskrhs"],
                "wihf": w["f"][0], "whhf": w["f"][1], "whh8f": w["f"][2],
                "bmf": w["f"][3],
                "wihb": w["b"][0], "whhb": w["b"][1], "whh8b": w["b"][2],
                "bmb": w["b"][3],
            }
        )

    nc = _get_nc(t_steps, V)
    res = None
    for attempt in range(3):
        try:
            res = run_bass_kernel_spmd(
                nc, in_maps, core_ids=list(range(NCORES)), trace=trace,
                **spmd_kwargs
            )
            break
        except Exception:
            # rare transient NRT_EXEC_UNIT_UNRECOVERABLE right after a
            # fresh NEFF load; a plain re-execute has always recovered
            if attempt == 2:
                raise
            import time as _time

            _time.sleep(2.0)

    out = np.empty((N, 2 * H), np.float32)
    for ci in range(NCORES):
        out[core_idx[ci], :H] = _unfold(res.results[ci]["hTf"])
        out[core_idx[ci], H:] = _unfold(res.results[ci]["hTb"])
    return out, res


def kernel(**inputs) -> np.ndarray:
    out, _ = _run(inputs)
    return out


# revision 20
# speedup vs baseline: 1.0154x; 1.0009x over previous
"""Trainium2 Bass kernel for nn_BiLSTM_7928509628689.

Masked bidirectional LSTM over N=2048 ragged sequences (T=64, D=512, H=256),
returning concat of final fwd/bwd hidden states [N, 2H].

Strategy (8 NeuronCores, data-parallel over N, 256 seqs/core):
  * Sequences are globally sorted by length (desc) and dealt round-robin to
    cores, so all cores carry a near-identical length profile. All
    sequences are right-aligned in time (they END at the last step), so at
    step s only the V_s longest sequences are active. V_s is baked into
    the program: every matmul / ACT / DVE op at step s is trimmed to V_s
    columns. Mean length is ~T/2, so this halves the PE columns.
  * All state kept TRANSPOSED: hT/cT [H, Ns] folded into persistent
    [128, 2*Ns] tiles updated in place (never-yet-active columns stay 0).
  * Per step and direction, gates^T [4H, V_s] are built in one 4-bank PSUM
    tile (bank order g,i,f,o) by one accumulation group per bank:
       4 matmuls  W_ih^T chunks @ x_s chunks    (input projection)
       2 matmuls  W_hh^T chunks @ hT halves     (recurrence)
       1 matmul   [b; mask_coef] @ [ones; maskinv_s]  (bias + pad forcing)
    Operands bf16 (fp32 PSUM accumulation), K=128 for every matmul so
    LDWEIGHTS stays FWL-pipelined under the stream.
  * Wide steps (V_s >= DRV) run the recurrence matmuls in fp8e4m3
    DoubleRow mode: one K=256 matmul replaces the two K=128 bf16 ones,
    halving the streamed columns of the h-projection. The hidden state is
    then stored as fp8 (h8) written directly by the DVE h-update; cell
    state and activations stay bf16, so only the recurrent matmul operand
    is quantized (validated ~1.2e-2 rel err vs 2e-2 budget).
  * Pad forcing: columns included before their sequence's first step get
    -40 on the i/f/o pre-activations, so their state is forced to ~0 until
    the sequence starts; the final state at the last step is exactly the
    masked-LSTM output for both directions (bwd consumes the time-reversed
    sequence).
  * ACT: one tanh over the g bank, one sigmoid spanning the i,f,o banks,
    one tanh(c); DVE does the elementwise updates on exact active ranges.

kernel(**inputs) takes the FULL unsharded inputs and returns [2048, 512] f32.
"""
import numpy as np

import concourse.tile as tile
from concourse import bacc, mybir
from concourse.bass_utils import run_bass_kernel_spmd
import bass_rust

F32 = mybir.dt.float32
BF16 = mybir.dt.bfloat16
F8 = mybir.dt.float8e4
DRPM = mybir.MatmulPerfMode.DoubleRow
AF = mybir.ActivationFunctionType
OP = mybir.AluOpType

N, T, D, H = 2048, 64, 512, 256
NCORES = 8
NS = N // NCORES           # 256 sequences per core
FH = 4 * H                 # 1024 gate rows
KD = D // 128              # 4 x-projection K chunks
KH = H // 128              # 2 h-projection K chunks
FORCE = -40.0              # gate penalty for pad steps
MB = 8                     # mask rhs block (steps per mask DMA)
DIRS = ("f", "b")
C_BF16 = True              # cell state dtype (bf16 enables DVE 4x mode)
DRV = 160                  # min V_s for fp8 DoubleRow recurrence matmuls

# merged packed-band PSUM slot layouts: 16 ranges of R in a 16R slot,
# ordered [g_f,g_b | i_f,i_b | f_f,f_b | o_f,o_b]; entries are
# (dir, m-chunk, offset) with m-chunk pairs in h-unit order. R=128 gives
# two parity slots (v<=128); R=64 gives four (v<=64), which lets pass1
# run two steps ahead without colliding with in-flight ACT reads.
def _mk_slots(R, nbank):
    order = []
    for g in (4, 0, 2, 6):  # g, i, f, o
        for d in ("f", "b"):
            order += [(d, g), (d, g + 1)]
    per = len(order) // nbank
    return [
        [(d, m, (b * per + j) * R) for j, (d, m) in
         enumerate(order[b * per : (b + 1) * per])]
        for b in range(nbank)
    ]


SLOT128 = _mk_slots(128, 4)
SLOT64 = _mk_slots(64, 2)
DOFF = {"f": 0, "b": 512}  # dir offset into shared c2/h2 state tiles
# PSUM bank order within the [128, 4*512] gates tile; sigmoid spans i,f,o
BANK_MS = ((4, 5), (0, 1), (2, 3), (6, 7))   # g, i, f, o
BANK_OF = [b * 512 for b in range(4)]

_NC_CACHE = {}


def _inst(r):
    return getattr(r, "ins", r)


def _build(t_steps, V):
    import contextlib

    nc = bacc.Bacc("TRN2", target_bir_lowering=False, debug=False)

    # first step using fp8 DoubleRow recurrence (V monotone nondecreasing)
    s_dr = next((s for s in range(t_steps) if int(V[s]) >= DRV), t_steps)

    x_dram = {}
    wih_d, whh_d, whh8_d, bm_d, out_d = {}, {}, {}, {}, {}
    for d in DIRS:
        # x stored [t, 128, KD, NS]: (p, k) <-> input dim  dd = KD*p + k
        x_dram[d] = nc.dram_tensor(
            f"x{d}", [t_steps, 128, KD, NS], BF16, kind="ExternalInput"
        ).ap()
        wih_d[d] = nc.dram_tensor(
            f"wih{d}", [128, KD, FH], BF16, kind="ExternalInput"
        ).ap()
        whh_d[d] = nc.dram_tensor(
            f"whh{d}", [128, KH, FH], BF16, kind="ExternalInput"
        ).ap()
        whh8_d[d] = nc.dram_tensor(
            f"whh8{d}", [128, KH, FH], F8, kind="ExternalInput"
        ).ap()
        bm_d[d] = nc.dram_tensor(f"bm{d}", [128, FH], BF16, kind="ExternalInput").ap()
        out_d[d] = nc.dram_tensor(
            f"hT{d}", [128, KH * NS], BF16, kind="ExternalOutput"
        ).ap()
    mask_d = nc.dram_tensor(
        "maskrhs", [128, t_steps * NS], BF16, kind="ExternalInput"
    ).ap()

    with tile.TileContext(nc) as tc:
        with contextlib.ExitStack() as ctx:
            wpool = ctx.enter_context(tc.tile_pool(name="w", bufs=1))
            xpool = ctx.enter_context(tc.tile_pool(name="x", bufs=4))
            mpool = ctx.enter_context(tc.tile_pool(name="mask", bufs=2))
            spool = ctx.enter_context(tc.tile_pool(name="state", bufs=1))
            opool = ctx.enter_context(tc.tile_pool(name="outs", bufs=1))
            apool = ctx.enter_context(tc.tile_pool(name="acts", bufs=2))
            pspool = ctx.enter_context(tc.tile_pool(name="ps", bufs=1, space="PSUM"))

            # spread weight loads across engine DMA queues so they land in
            # parallel with the first x tiles (which use the sync queue);
            # wih is split by gate group so the first bank's weights (g)
            # arrive first and step 0 can start early
            wq = {"f": nc.scalar, "b": nc.gpsimd}
            wih_t, whh_t, whh8_t, bm_t = {}, {}, {}, {}
            for d in DIRS:
                wih_t[d] = wpool.tile([128, KD, FH], BF16, tag=f"wih_{d}", name=f"wih_{d}")
                for gm in (4, 0, 2, 6):  # bank order: g, i, f, o
                    gsl = slice(gm * 128, (gm + 2) * 128)
                    wq[d].dma_start(wih_t[d][:, :, gsl], wih_d[d][:, :, gsl])
                bm_t[d] = wpool.tile([128, FH], BF16, tag=f"bm_{d}", name=f"bm_{d}")
                wq[d].dma_start(bm_t[d][:], bm_d[d][:])
            for d in DIRS:
                whh_t[d] = wpool.tile([128, KH, FH], BF16, tag=f"whh_{d}", name=f"whh_{d}")
                wq[d].dma_start(whh_t[d][:], whh_d[d][:])
                whh8_t[d] = wpool.tile([128, KH, FH], F8, tag=f"whh8_{d}", name=f"whh8_{d}")
                wq[d].dma_start(whh8_t[d][:], whh8_d[d][:])

            # persistent state tiles, updated in place; inactive columns
            # stay zero from this init. Both dirs share one tile so the
            # packed band can run cross-dir elementwise ops on one view.
            # state memsets go to gpsimd (idle engine) so the vector
            # engine's first op is the warmup-weight memset the PE waits on
            CDT = BF16 if C_BF16 else F32
            h2 = spool.tile([128, 2 * KH * NS], BF16, tag="h2", name="h2")
            nc.gpsimd.memset(h2[:], 0.0)
            c2 = spool.tile([128, 2 * KH * NS], CDT, tag="c2", name="c2")
            nc.gpsimd.memset(c2[:], 0.0)
            # fp8 copy of the hidden state for DoubleRow recurrence steps
            h8 = spool.tile([128, 2 * KH * NS], F8, tag="h8", name="h8")
            nc.gpsimd.memset(h8[:], 0.0)

            def h8v(d, v):
                return h8[:, DOFF[d] : DOFF[d] + 2 * NS].rearrange(
                    "p (two n) -> p two n", two=2
                )[:, :, :v]

            # single shared gates tile: unpacked steps use halves per dir,
            # packed steps use parity-alternating merged 2048 slots
            PS = pspool.tile([128, 2 * 4 * 512], F32, tag="ps", name="ps")
            ps_t = {"f": PS[:, 0:2048], "b": PS[:, 2048:4096]}

            # PE warm-up burst: dense dummy matmuls during the initial
            # weight/x DMA window so HAM reaches full clock before step 0.
            wrm = wpool.tile([128, 512], BF16, tag="warm", name="warm")
            nc.vector.memset(wrm[:], 0.0)
            NWARM = 28
            for i in range(NWARM):
                nc.tensor.matmul(
                    PS[:, 0:512], wrm[:, 0:128], wrm[:],
                    start=(i == 0), stop=(i == NWARM - 1),
                )

            # --- step scheduling helpers -------------------------------
            xts_q, mt_q = {}, {}

            def fetch(s2):
                """Issue x/mask DMAs for step s2 (idempotent)."""
                if s2 >= t_steps or s2 in xts_q:
                    return
                blk = s2 // MB
                if blk not in mt_q:
                    mw = min(MB, t_steps - blk * MB) * NS
                    mt = mpool.tile([128, MB * NS], BF16, tag="m", name="mtile")
                    nc.sync.dma_start(
                        mt[:, :mw], mask_d[:, blk * MB * NS : blk * MB * NS + mw]
                    )
                    mt_q[blk] = mt
                    for old in [b for b in mt_q if b < blk - 1]:
                        del mt_q[old]
                v = int(V[s2])
                xts = {}
                for d in DIRS:
                    xts[d] = xpool.tile(
                        [128, KD, NS], BF16, tag=f"x_{d}", name=f"x_{d}"
                    )
                    nc.sync.dma_start(xts[d][:, :, :v], x_dram[d][s2][:, :, :v])
                xts_q[s2] = xts

            def vvq(ap, q, v):
                return ap.rearrange("p (q n) -> p q n", q=q)[:, :, :v]

            def mrhs_of(s, v):
                mt = mt_q[s // MB]
                return mt[:, (s % MB) * NS : (s % MB) * NS + v]

            def playout(s):
                v = int(V[s])
                return v, 128, 2048 * (s % 2), SLOT128

            def p1_packed(s):
                """x-projection + bias into the parity slot of step s."""
                v, R, base, slots = playout(s)
                xts = xts_q[s]
                mrhs = mrhs_of(s, v)
                for bank in slots:
                    start_mm = None
                    for idx, (d, m, off) in enumerate(bank):
                        o_ap = PS[:, base + off : base + off + v]
                        msl = slice(m * 128, (m + 1) * 128)
                        r = nc.tensor.matmul(
                            o_ap, wih_t[d][:, 0, msl], xts[d][:, 0, :v],
                            start=(idx == 0), stop=False,
                        )
                        if idx == 0:
                            start_mm = _inst(r)
                        else:
                            bass_rust.add_dep_helper(
                                _inst(r), start_mm, sync=False,
                                reason="psum bank group order",
                            )
                        for k in range(1, KD):
                            nc.tensor.matmul(
                                o_ap, wih_t[d][:, k, msl], xts[d][:, k, :v],
                                start=False, stop=False,
                            )
                        nc.tensor.matmul(
                            o_ap, bm_t[d][:, msl], mrhs,
                            start=False, stop=False,
                        )

            def p2_act_packed(s):
                """recurrent projection + merged cross-dir ACT/DVE of step s."""
                v, R, base, slots = playout(s)

                def v2(ap, q):
                    return vvq(ap, q, v)

                for bank in slots:
                    for idx, (d, m, off) in enumerate(bank):
                        o_ap = PS[:, base + off : base + off + v]
                        msl = slice(m * 128, (m + 1) * 128)
                        for kk in range(KH):
                            nc.tensor.matmul(
                                o_ap,
                                whh_t[d][:, kk, msl],
                                h2[:, DOFF[d] + kk * NS : DOFF[d] + kk * NS + v],
                                start=False,
                                stop=(idx == len(bank) - 1 and kk == KH - 1),
                            )

                # cross-dir elementwise: ranges of R ordered
                # [x_f0, x_f1, x_b0, x_b1] per gate, h-unit aligned with
                # the 256-wide ranges of the shared c2/h2 state tiles.
                # The sigmoid is split if/o: sig_if only waits on the i/f
                # banks' recurrence matmuls (the o bank finishes ~200ns
                # later) and is 4 ranges shorter, so the c-update chain
                # starts earlier; sig_o runs in the ACT idle window while
                # the DVE does the cell update, well before hmul needs it.
                tg = apool.tile([128, 4 * R], BF16, tag=f"tg2_{R}", name="tg2")
                nc.scalar.activation(
                    v2(tg[:], 4), v2(PS[:, base : base + 4 * R], 4), AF.Tanh
                )
                si = apool.tile([128, 12 * R], BF16, tag=f"si2_{R}", name="si2")
                nc.scalar.activation(
                    v2(si[:, 0 : 8 * R], 8),
                    v2(PS[:, base + 4 * R : base + 12 * R], 8),
                    AF.Sigmoid,
                )
                nc.scalar.activation(
                    v2(si[:, 8 * R : 12 * R], 4),
                    v2(PS[:, base + 12 * R : base + 16 * R], 4),
                    AF.Sigmoid,
                )
                # cmul first: it only needs sig_if, while t1 also waits on
                # tanh_g
                nc.vector.tensor_tensor(
                    v2(c2[:], 4), v2(si[:, 4 * R : 8 * R], 4), v2(c2[:], 4), OP.mult
                )
                t1 = apool.tile([128, 4 * R], BF16, tag=f"t12_{R}", name="t12")
                nc.vector.tensor_tensor(
                    v2(t1[:], 4), v2(si[:, 0 : 4 * R], 4), v2(tg[:], 4), OP.mult
                )
                nc.vector.tensor_tensor(
                    v2(c2[:], 4), v2(c2[:], 4), v2(t1[:], 4), OP.add
                )
                tcn = apool.tile([128, 4 * R], BF16, tag=f"tc2_{R}", name="tc2")
                nc.scalar.activation(v2(tcn[:], 4), v2(c2[:], 4), AF.Tanh)
                nc.vector.tensor_tensor(
                    v2(h2[:], 4), v2(si[:, 8 * R : 12 * R], 4), v2(tcn[:], 4), OP.mult
                )
                if s + 1 >= s_dr:
                    # next step's recurrence runs in fp8 DoubleRow: refresh
                    # the fp8 state copy (cheap DVE copy via scalar engine
                    # stays off the critical path; use DVE mult-by-one? a
                    # plain tensor_tensor copy is simplest)
                    nc.vector.tensor_tensor(
                        v2(h8[:], 4), v2(h2[:], 4), v2(h2[:], 4), OP.bypass
                    )
                del xts_q[s]

            def band(s):
                return 2 if int(V[s]) <= 128 else 3

            # --- main loop: the packed prefix is software-pipelined.
            # pass1 runs ahead of the serial ACT/DVE chain: two steps in
            # the 4-parity v<=64 band, one step in the 2-parity band
            # (deeper lookahead there would WAR-collide with in-flight
            # ACT reads). The unpacked suffix overlaps via dir order. ----
            next_p1 = 0
            for s in range(t_steps):
                fetch(s)
                fetch(s + 1)
                fetch(s + 2)
                v = int(V[s])
                last = s == t_steps - 1
                use_dr = s >= s_dr and s > 0

                def v2(ap, q):
                    return vvq(ap, q, v)

                if band(s) != 3:
                    assert not last
                    while next_p1 <= s:
                        p1_packed(next_p1)
                        next_p1 += 1
                    p2_act_packed(s)
                    fetch(s + 3)
                    while (
                        next_p1 < t_steps
                        and band(next_p1) != 3
                        and next_p1 <= s + (2 if band(next_p1) == 1 else 1)
                    ):
                        p1_packed(next_p1)
                        next_p1 += 1
                    continue

                xts = xts_q[s]
                mrhs = mrhs_of(s, v)
                for d in DIRS:
                    xt = xts[d]
                    ps = ps_t[d]
                    cc = c2[:, DOFF[d] : DOFF[d] + 512]
                    hh = h2[:, DOFF[d] : DOFF[d] + 512]

                    banks = [
                        [(b * 512 + half * NS, BANK_MS[b][half]) for half in range(2)]
                        for b in range(4)
                    ]
                    tg_sl = ps[:, 0:512]
                    sifo_sl = ps[:, 512 : 4 * 512]

                    # pass 1: x-projection + bias/mask (independent of h),
                    # one accumulation group per physical bank
                    for regs in banks:
                        start_mm = None
                        for idx, (off, m) in enumerate(regs):
                            o_ap = ps[:, off : off + v]
                            msl = slice(m * 128, (m + 1) * 128)
                            r = nc.tensor.matmul(
                                o_ap, wih_t[d][:, 0, msl], xt[:, 0, :v],
                                start=(idx == 0), stop=False,
                            )
                            if idx == 0:
                                start_mm = _inst(r)
                            else:
                                # later regions rely on the bank-wide
                                # has_written clear done by the start matmul
                                bass_rust.add_dep_helper(
                                    _inst(r), start_mm, sync=False,
                                    reason="psum bank group order",
                                )
                            for k in range(1, KD):
                                nc.tensor.matmul(
                                    o_ap, wih_t[d][:, k, msl], xt[:, k, :v],
                                    start=False, stop=False,
                                )
                            nc.tensor.matmul(
                                o_ap, bm_t[d][:, msl], mrhs,
                                start=False, stop=False,
                            )

                    # pass 2: recurrent projection last, so the PE only
                    # stalls on h right before the gates complete
                    for regs in banks:
                        for idx, (off, m) in enumerate(regs):
                            o_ap = ps[:, off : off + v]
                            msl = slice(m * 128, (m + 1) * 128)
                            if use_dr:
                                nc.tensor.matmul(
                                    o_ap, whh8_t[d][:, :, msl], h8v(d, v),
                                    start=False, stop=(idx == len(regs) - 1),
                                    perf_mode=DRPM,
                                )
                            else:
                                for kk in range(KH):
                                    nc.tensor.matmul(
                                        o_ap,
                                        whh_t[d][:, kk, msl],
                                        h2[:, DOFF[d] + kk * NS : DOFF[d] + kk * NS + v],
                                        start=False,
                                        stop=(idx == len(regs) - 1 and kk == KH - 1),
                                    )

                    # ACT/DVE on strided multi-range views that skip the
                    # dead gaps between half-ranges
                    tg = apool.tile([128, 512], BF16, tag=f"tg_{d}", name=f"tg_{d}")
                    nc.scalar.activation(v2(tg[:], 2), v2(tg_sl, 2), AF.Tanh)
                    # one sigmoid spanning i, f, o; out layout is 6 ranges of
                    # 256 (i0,i1,f0,f1,o0,o1)
                    si = apool.tile([128, 3 * 512], BF16, tag=f"si_{d}", name=f"si_{d}")
                    nc.scalar.activation(v2(si[:], 6), v2(sifo_sl, 6), AF.Sigmoid)

                    t1 = apool.tile([128, 512], BF16, tag=f"t1_{d}", name=f"t1_{d}")
                    nc.vector.tensor_tensor(
                        v2(t1[:], 2), v2(si[:, 0:512], 2), v2(tg[:], 2), OP.mult
                    )
                    nc.vector.tensor_tensor(
                        v2(cc, 2), v2(si[:, 512:1024], 2), v2(cc, 2), OP.mult
                    )
                    nc.vector.tensor_tensor(
                        v2(cc, 2), v2(cc, 2), v2(t1[:], 2), OP.add
                    )
                    tcn = apool.tile([128, 512], BF16, tag=f"tc_{d}", name=f"tc_{d}")
                    nc.scalar.activation(v2(tcn[:], 2), v2(cc, 2), AF.Tanh)
                    if last:
                        hf = opool.tile([128, 512], BF16, tag=f"hout_{d}", name=f"hout_{d}")
                        nc.vector.tensor_tensor(hf[:], si[:, 1024:1536], tcn[:], OP.mult)
                        nc.sync.dma_start(out_d[d][:], hf[:])
                    elif s + 1 >= s_dr:
                        # next step reads the fp8 state: write h8 directly
                        nc.vector.tensor_tensor(
                            vvq(h8[:, DOFF[d] : DOFF[d] + 512], 2, v),
                            v2(si[:, 1024:1536], 2), v2(tcn[:], 2), OP.mult
                        )
                    else:
                        nc.vector.tensor_tensor(
                            v2(hh, 2), v2(si[:, 1024:1536], 2), v2(tcn[:], 2), OP.mult
                        )

    nc.compile()
    return nc


def _get_nc(t_steps, V):
    key = (t_steps, tuple(V))
    if key not in _NC_CACHE:
        _NC_CACHE[key] = _build(t_steps, V)
    return _NC_CACHE[key]


def _prep_weights(W_ih, W_hh, b):
    """lhsT layouts for one direction."""
    import ml_dtypes

    wdt = ml_dtypes.bfloat16
    wih = np.ascontiguousarray(
        W_ih.T.reshape(128, KD, FH).astype(wdt)
    )  # (p, k) <-> dd = KD*p + k
    whh = np.ascontiguousarray(
        W_hh.T.reshape(KH, 128, FH).transpose(1, 0, 2).astype(wdt)
    )  # (p, kk) <-> hrow = 128*kk + p
    whh8 = np.ascontiguousarray(
        whh.astype(np.float32).astype(ml_dtypes.float8_e4m3fn)
    )
    coef = np.zeros(FH, np.float32)
    coef[: 2 * H] = FORCE       # i, f gates
    coef[3 * H :] = FORCE       # o gate
    bm = np.zeros((128, FH), np.float32)
    bm[0] = b.astype(np.float32)
    bm[1] = coef
    bm = np.ascontiguousarray(bm.astype(wdt))
    return wih, whh, whh8, bm


def _prep_core(seqs_c, lens_c, t_steps):
    """Per-core device arrays. seqs_c [NS, T, D], lens_c [NS] (sorted desc)."""
    import ml_dtypes

    bf16 = ml_dtypes.bfloat16
    ns = seqs_c.shape[0]
    shift = t_steps - lens_c  # pad steps per sequence
    src_t = np.arange(t_steps)[None, :] - shift[:, None]      # [NS, t]
    valid = src_t >= 0
    gat = seqs_c[np.arange(ns)[:, None], np.clip(src_t, 0, T - 1)]
    xf = np.where(valid[..., None], gat, np.float32(0.0))     # right-aligned
    xb = seqs_c[:, t_steps - 1 :: -1, :]                      # time-reversed

    def to_dev(x_ntd):
        # [NS, t, D] -> [t, 128, KD, NS] with dd = KD*p + k
        xt = x_ntd.transpose(1, 2, 0).astype(bf16)            # [t, D, NS]
        return np.ascontiguousarray(xt.reshape(t_steps, 128, KD, ns))

    maskinv = (np.arange(t_steps)[:, None] < shift[None, :]).astype(np.float32)
    maskrhs = np.zeros((128, t_steps * ns), np.float32)
    maskrhs[0] = 1.0
    maskrhs[1] = maskinv.reshape(t_steps * ns)
    maskrhs = np.ascontiguousarray(maskrhs.astype(bf16))
    return {"xf": to_dev(xf), "xb": to_dev(xb), "maskrhs": maskrhs}


def _unfold(hT):
    """[128, KH*NS] device tile -> [NS, H] h matrix."""
    hT = np.asarray(hT, dtype=np.float32)
    h_rows = np.concatenate([hT[:, i * NS : (i + 1) * NS] for i in range(KH)], axis=0)
    return h_rows.T  # [NS, H]


def _run(inputs, trace=False, t_cap=None, **spmd_kwargs):
    import ml_dtypes

    all_embs = np.asarray(inputs["all_embs"], dtype=np.float32)
    lengths = np.asarray(inputs["lengths"]).astype(np.int64)
    starts = np.asarray(inputs["starts"]).astype(np.int64)

    if np.array_equal(starts, np.arange(N, dtype=np.int64) * T):
        seqs = all_embs.reshape(N, T, D)
    else:
        seqs = all_embs[starts[:, None] + np.arange(T)[None, :]]

    # global sort by length desc, deal round-robin to cores
    order = np.argsort(-lengths, kind="stable")
    t_steps = int(lengths.max())
    if t_cap is not None:
        t_steps = min(t_steps, t_cap)
    core_idx = [order[c::NCORES] for c in range(NCORES)]  # [NCORES][NS]

    # baked active widths: V_s = max over cores of #{len >= t_steps - s}
    Ls = np.stack([np.minimum(lengths[ci], t_steps) for ci in core_idx])  # [NC, NS]
    thr = t_steps - np.arange(t_steps)  # [t]
    V = (Ls[:, None, :] >= thr[None, :, None]).sum(-1).max(0)  # [t]
    V = np.maximum(V, 1)

    w = {}
    for d, (wi, wh, bb) in {
        "f": (inputs["W_ih_f"], inputs["W_hh_f"], inputs["b_f"]),
        "b": (inputs["W_ih_b"], inputs["W_hh_b"], inputs["b_b"]),
    }.items():
        w[d] = _prep_weights(
            np.asarray(wi, np.float32), np.asarray(wh, np.float32),
            np.asarray(bb, np.float32),
        )

    in_maps = []
    for ci in range(NCORES):
        idx = core_idx[ci]
        m = _prep_core(seqs[idx], np.minimum(lengths[idx], t_steps), t_steps)
        in_maps.append(
            {
                "xf": m["xf"], "xb": m["xb"], "maskrhs": m["maskrhs"],
                "wihf": w["f"][0], "whhf": w["f"][1], "whh8f": w["f"][2],
                "bmf": w["f"][3],
                "wihb": w["b"][0], "whhb": w["b"][1], "whh8b": w["b"][2],
                "bmb": w["b"][3],
            }
        )

    nc = _get_nc(t_steps, V)
    res = None
    for attempt in range(3):
        try:
            res = run_bass_kernel_spmd(
                nc, in_maps, core_ids=list(range(NCORES)), trace=trace,
                **spmd_kwargs
            )
            break
        except Exception:
            # rare transient NRT_EXEC_UNIT_UNRECOVERABLE right after a
            # fresh NEFF load; a plain re-execute has always recovered
            if attempt == 2:
                raise
            import time as _time

            _time.sleep(2.0)

    out = np.empty((N, 2 * H), np.float32)
    for ci in range(NCORES):
        out[core_idx[ci], :H] = _unfold(res.results[ci]["hTf"])
        out[core_idx[ci], H:] = _unfold(res.results[ci]["hTb"])
    return out, res


def kernel(**inputs) -> np.ndarray:
    out, _ = _run(inputs)
    return out
